# revision 12
# baseline (speedup 1.0000x reference)
"""AdaLoRA routed-LoRA kernel for 8 Trainium2 NeuronCores (v5).

Problem (nn_AdaLoRA): per token t with expert index i:
    ds[t, :]  = slots[t, :] @ down_table[i]            # [1024] @ [1024, 16]
    out[t, :] = (ds[t, :] @ up_table[i]) / sqrt(16)    # [16] @ [16, 1024]

Sharding: data-parallel over batch (B=8 -> one batch row per core; LoRA
tables replicated). Per core: 256 tokens = 2 tiles of 128 tokens.

History: 91.8 -> 69.0 (v3, int8 tables + in-flight f16 cast) -> 64.3
(v4, per-c gathers + dch bufs=8 + interleaved order) -> v5.

v5 -> v6, from the v5 NTFF profile: the tail chain is window-end(52us)
-> last-group MMs -> out copy -> out DMA -> closing. v6: splits the
last up-gather into column halves so its MMs overlap the drain, splits
the first down op for an earlier window start, rebalances the last
chunks' ranks (ACT-queue-end gates fin(1,3)), folds cs into the
constant blob as f16, and runs the final out copy on ACT+DVE in
parallel.

v4 -> v5, from the v4 NTFF profile:
- v4's M=32 lhsT slices disabled FWL (fast weight load needs a full
  128-col stationary) -> half the up-MMs ran ~620ns instead of 379.
  v5 reverts to full [128,128] masked lhsT per (c, g) (all MMs 379ns)
  but keeps the replicate cheap by materializing the broadcast rhs
  with a tiny DVE copy [4,128]->[4,512]; v3's stride-0 rhs on the PE
  moving operand made those reps ~1.25us each.
- ACT/DVE accumulator outputs now write f16 ds16 columns directly
  (allow_low_precision; the engine accumulator is f32 internally and
  we cast to f16 anyway) - removes 2 DVE copies per chunk and the
  DVE-waits-on-ACT serialization that idled DVE ~4us in v4.
- MM emission reordered so a late lhsT (c3, built from the last-
  arriving down chunk) never blocks other groups' data-ready MMs:
  per tile, g0/g1 run c0..c2 first, c3 MMs join when ready, then
  g2/g3 chase their gather arrivals.
- Constants (ident/m4g/e4) load as one blob DMA.

HW findings baked into this design (CoreSim/TimelineSim disagree!):
- multi-offset indirect DMA (offset AP [128,k>1]) returns garbage on
  real HW although CoreSim models it fine -> single-offset only.
- tensor_tensor_reduce faults the device -> use scalar_tensor_tensor.
- gpsimd tensor ops cannot read PSUM (BIR verifier).
- tile-pool buffer rings gate SWDGE descriptor *generation*: too few
  gather buffers stall the whole DMA queue mid-stream.
- DVE 2x perf mode needs all-16-bit operands (int8 operands drop DVE
  to 1x; ACT is 1x regardless) -> tables must land f16; fp8 up-table
  fails numerically (3.4e-2 vs 2e-2 gate, host-sim).
- FWL needs a full 128-col stationary operand: lhsT slices narrower
  than 128 make MMs ~1.6x slower.
"""

import numpy as np

B, K, DIM, RANK, NE = 8, 256, 1024, 16, 4096
ROW = DIM * RANK  # 16384 int8 elements per down-table row
SCALE = 1.0 / 4.0  # 1/sqrt(RANK)
P = 128
N_TILE = K // P  # 2 token tiles per core
RSLOT = 4  # ranks per partition in the up gather
TPG = P // RSLOT  # 32 tokens per up group
NGRP = P // TPG  # 4 up groups per tile
N_CORES = 8
CBLOB = P + NGRP * P + P  # ident | m4g | e4pad columns

_CACHE = {}


def _build():
    from concourse import bacc, bass, mybir, tile

    f32 = mybir.dt.float32
    f16 = mybir.dt.float16
    i8 = mybir.dt.int8
    i32 = mybir.dt.int32
    mult = mybir.AluOpType.mult
    Copy = mybir.ActivationFunctionType.Copy

    nc = bacc.Bacc("TRN2", target_bir_lowering=False, dynamic_dma_scratch_size=65536)
    # idxcat[:, 0:2] = down row idx per (p, t); [:, 2:10] = up4 row idx per (p, t*4+g)
    idxcat = nc.declare_dram_parameter("idxcat", [P, 2 + N_TILE * NGRP], i32, isOutput=False)
    slots = nc.declare_dram_parameter("slots", [K, DIM], f16, isOutput=False)
    cs2 = nc.declare_dram_parameter("cs2", [P, N_TILE], f32, isOutput=False)
    down = nc.declare_dram_parameter("down", [NE, ROW], i8, isOutput=False)
    up4 = nc.declare_dram_parameter("up4", [NE * RSLOT, RSLOT * DIM], i8, isOutput=False)
    cblob = nc.declare_dram_parameter("cblob", [P, CBLOB], f16, isOutput=False)
    out = nc.declare_dram_parameter("out", [K, DIM], f16, isOutput=True)

    with tile.TileContext(nc) as tc:
        with (
            tc.tile_pool(name="io", bufs=2) as io_pool,
            tc.tile_pool(name="gath", bufs=6) as gpool,
            tc.tile_pool(name="upg", bufs=8) as upool,
            tc.tile_pool(name="prod", bufs=4) as ppool,
            tc.tile_pool(name="misc", bufs=1) as mpool,
            tc.tile_pool(name="ds", bufs=8) as dspool,
            tc.tile_pool(name="psT", bufs=2, space="PSUM") as psT,
            tc.tile_pool(name="psR", bufs=2, space="PSUM") as psR,
            tc.tile_pool(name="psO", bufs=2, space="PSUM") as psO,
        ):
            # ---- index load first: it gates every gather ----
            idx_sb = mpool.tile([P, 2 + N_TILE * NGRP], i32)
            nc.sync.dma_start(out=idx_sb[:], in_=idxcat[:, :])

            # ---- remaining loads on the second HWDGE ring (ACT) ----
            slots_all = mpool.tile([P, N_TILE, DIM], f16)
            nc.scalar.dma_start(
                out=slots_all[:], in_=slots[:, :].rearrange("(t p) d -> p t d", p=P)
            )
            cb = mpool.tile([P, CBLOB], f16)
            nc.scalar.dma_start(out=cb[:], in_=cblob[:, :])
            ident = cb[:, 0:P]
            m4g = cb[:, P : P + NGRP * P]
            e4_sb = cb[0:RSLOT, P + NGRP * P : P + NGRP * P + P]
            cs_sb = mpool.tile([P, N_TILE], f32)
            nc.scalar.dma_start(out=cs_sb[:], in_=cs2[:, :])

            # ---- indirect gathers: all issued up front on the SWDGE queue ----
            dch = {}
            upc = {}

            def emit_down(t, c, half=None):
                # half=0/1 gathers 2 of the 4 rank-rows (earlier first data)
                nr = RSLOT if half is None else RSLOT // 2
                d = gpool.tile([P, nr, DIM], f16, tag="dch")
                nc.gpsimd.indirect_dma_start(
                    out=d[:].rearrange("p r d -> p (r d)"),
                    out_offset=None,
                    in_=down[:],
                    in_offset=bass.IndirectOffsetOnAxis(ap=idx_sb[:, t : t + 1], axis=0),
                    element_offset=c * RSLOT * DIM + (0 if half is None else half * nr * DIM),
                )
                if half is None:
                    dch[t, c] = d
                else:
                    dch[t, c, half] = d

            def emit_up(t, g, halves=1):
                u = upool.tile([P, RSLOT * DIM], f16, tag="upc")
                w = RSLOT * DIM // halves
                for h in range(halves):
                    nc.gpsimd.indirect_dma_start(
                        out=u[:, h * w : (h + 1) * w],
                        out_offset=None,
                        in_=up4[:],
                        in_offset=bass.IndirectOffsetOnAxis(
                            ap=idx_sb[:, 2 + t * NGRP + g : 3 + t * NGRP + g], axis=0
                        ),
                        element_offset=h * w,
                    )
                upc[t, g] = u

            for c in range(RSLOT):
                emit_down(0, c)
            emit_down(1, 0)
            emit_down(1, 1)
            emit_up(0, 0)
            emit_up(0, 1)
            emit_up(0, 2)
            emit_down(1, 2)
            emit_up(0, 3)
            emit_down(1, 3)
            emit_up(1, 0)
            emit_up(1, 1)
            emit_up(1, 2)
            emit_up(1, 3, halves=2)

            lhsT_all = mpool.tile([P, N_TILE, RSLOT, NGRP, P], f16)
            scr_act = mpool.tile([P, DIM], f16)
            scr_dve = mpool.tile([P, DIM], f16)

            # ---- down projection per (t, c): 4 ranks {4rp+c} ----
            N_ACT = {  # rank-slots on the ACT path per (t, c)
                (0, 0): 3, (0, 1): 3, (0, 2): 3, (0, 3): 2,
                (1, 0): 3, (1, 1): 3, (1, 2): 3, (1, 3): 1,
            }

            def emit_ranks(t, c):
                na = N_ACT[t, c]

                def dsrc(rp):
                    if (t, c) in dch:
                        return dch[t, c][:, rp, :]
                    h = rp // (RSLOT // 2)
                    return dch[t, c, h][:, rp % (RSLOT // 2), :]
                ds16 = dspool.tile([P, RSLOT], f16, tag="ds16")
                with nc.allow_low_precision(reason="accumulator is f32; f16 on writeout"):
                    for rp in range(na):  # ACT path first: feed ACT asap
                        prod = ppool.tile([P, DIM], f16, tag="prod")
                        nc.vector.tensor_tensor(
                            out=prod[:], in0=slots_all[:, t, :], in1=dsrc(rp), op=mult
                        )
                        nc.scalar.activation(
                            out=scr_act[:],
                            in_=prod[:],
                            func=Copy,
                            accum_out=ds16[:, rp : rp + 1],
                        )
                    for rp in range(na, RSLOT):
                        nc.vector.scalar_tensor_tensor(
                            out=scr_dve[:],
                            in0=slots_all[:, t, :],
                            scalar=1.0,
                            in1=dsrc(rp),
                            op0=mult,
                            op1=mult,
                            accum_out=ds16[:, rp : rp + 1],
                        )
                return ds16

            def emit_fin(t, c, ds16):
                # ds16 [tok, rp] -> dsT [rp, tok] -> dsT4 [rp, (g,tok)] ->
                # rep[p, (g,m)] = dsT[p%4, m] -> mask [p//4 == m%32, g-match]
                dsT_psum = psT.tile([RSLOT, P], f16, space="PSUM", tag="dsT")
                nc.tensor.transpose(out=dsT_psum[:], in_=ds16[:], identity=ident)
                dsT4 = dspool.tile([RSLOT, NGRP, P], f16, tag="dsT4")
                nc.vector.tensor_copy(
                    out=dsT4[:],
                    in_=dsT_psum[:]
                    .rearrange("q (one c) -> q one c", one=1)
                    .broadcast_to((RSLOT, NGRP, P)),
                )
                rep = psR.tile([P, NGRP * P], f32, space="PSUM", tag="rep")
                nc.tensor.matmul(
                    out=rep[:],
                    lhsT=e4_sb,
                    rhs=dsT4[:].rearrange("q g c -> q (g c)"),
                    start=True,
                    stop=True,
                )
                nc.vector.tensor_tensor(
                    out=lhsT_all[:, t, c, :, :].rearrange("p g c -> p (g c)"),
                    in0=rep[:],
                    in1=m4g,
                    op=mult,
                )

            out_psum = {}
            n_mm = {}
            for t in range(N_TILE):
                op_t = psO.tile([P, DIM], f32, space="PSUM", tag="outp")
                out_psum[t] = op_t
                n_mm[t, 0] = 0
                n_mm[t, 1] = 0

            def emit_mm(t, g, c, ns=(0, 1)):
                for n in ns:
                    n0, n1 = n * 512, (n + 1) * 512
                    n_mm[t, n] += 1
                    nc.tensor.matmul(
                        out=out_psum[t][:, n0:n1],
                        lhsT=lhsT_all[:, t, c, g, :],
                        rhs=upc[t, g][:, c * DIM + n0 : c * DIM + n1],
                        start=(n_mm[t, n] == 1),
                        stop=(n_mm[t, n] == NGRP * RSLOT),
                    )

            def emit_out(t, dve_half=False):
                out_sb = io_pool.tile([P, DIM], f16, tag="osb")
                for h in range(2):
                    h0, h1 = h * 512, (h + 1) * 512
                    if dve_half and h == 1:
                        nc.vector.tensor_scalar(
                            out=out_sb[:, h0:h1],
                            in0=out_psum[t][:, h0:h1],
                            scalar1=cs_sb[:, t : t + 1],
                            scalar2=None,
                            op0=mult,
                        )
                    else:
                        nc.scalar.activation(
                            out=out_sb[:, h0:h1],
                            in_=out_psum[t][:, h0:h1],
                            func=Copy,
                            scale=cs_sb[:, t : t + 1],
                        )
                    nc.sync.dma_start(
                        out=out[t * P : (t + 1) * P, h0:h1], in_=out_sb[:, h0:h1]
                    )

            # ---- emission in expected-arrival order ----
            # Rules learned from the v4-v7 profiles: (1) every MM must be
            # emitted AFTER the fin that writes its lhsT (deps only look
            # backward); (2) DVE rank-ops come before pending fin-ops so ACT
            # never starves for prods; (3) each fin's PE transpose/rep sits at
            # the PE-queue position matching when its ds16 lands, never ahead
            # of data-ready MMs; (4) the last group's MMs run n0-before-n1 so
            # the first out-half's DMA overlaps the second half's MMs.
            ds = {}
            ds[0, 0] = emit_ranks(0, 0)
            ds[0, 1] = emit_ranks(0, 1)
            emit_fin(0, 0, ds[0, 0])
            ds[0, 2] = emit_ranks(0, 2)
            emit_fin(0, 1, ds[0, 1])
            ds[0, 3] = emit_ranks(0, 3)
            emit_fin(0, 2, ds[0, 2])
            ds[1, 0] = emit_ranks(1, 0)

            emit_mm(0, 0, 0)
            emit_mm(0, 0, 1)
            emit_mm(0, 1, 0)
            emit_mm(0, 1, 1)
            emit_fin(0, 3, ds[0, 3])
            ds[1, 1] = emit_ranks(1, 1)
            emit_mm(0, 0, 2)
            emit_mm(0, 1, 2)
            emit_mm(0, 0, 3)
            emit_mm(0, 1, 3)
            emit_fin(1, 0, ds[1, 0])
            ds[1, 2] = emit_ranks(1, 2)
            for c in range(RSLOT):
                emit_mm(0, 2, c)
            ds[1, 3] = emit_ranks(1, 3)
            emit_mm(0, 3, 0)
            emit_mm(0, 3, 1)
            emit_fin(1, 1, ds[1, 1])
            emit_mm(0, 3, 2)
            emit_mm(0, 3, 3)
            emit_out(0)

            emit_mm(1, 0, 0)
            emit_mm(1, 0, 1)
            emit_fin(1, 2, ds[1, 2])
            emit_mm(1, 0, 2)
            emit_mm(1, 1, 0)
            emit_mm(1, 1, 1)
            emit_fin(1, 3, ds[1, 3])
            emit_mm(1, 1, 2)
            emit_mm(1, 0, 3)
            emit_mm(1, 1, 3)
            for c in range(RSLOT):
                emit_mm(1, 2, c)
            for c in range(RSLOT):
                emit_mm(1, 3, c, ns=(0,))
            for c in range(RSLOT):
                emit_mm(1, 3, c, ns=(1,))
            emit_out(1, dve_half=True)
    nc.compile()
    return nc


def _get_nc():
    if "nc" not in _CACHE:
        _CACHE["nc"] = _build()
    return _CACHE["nc"]


def _prep_in_maps(slots, indices, down_proj_values, up_proj_values):
    slots = np.ascontiguousarray(np.asarray(slots, dtype=np.float32).astype(np.float16))
    indices = np.ascontiguousarray(np.asarray(indices).astype(np.int32))
    downT = np.asarray(down_proj_values, dtype=np.float32).transpose(0, 2, 1)  # [NE,R,D]
    up = np.asarray(up_proj_values, dtype=np.float32)  # [NE,R,D]

    # per-expert int8 quantization
    s_d = np.abs(downT).max(axis=(1, 2)) / 127.0  # [NE]
    s_u = np.abs(up).max(axis=(1, 2)) / 127.0
    # rank order (c, rp): rank r = 4*rp + c at block c*4096 + rp*1024
    perm = np.array([4 * rp + c for c in range(RSLOT) for rp in range(RSLOT)])
    down_q = np.ascontiguousarray(
        np.clip(np.round(downT[:, perm, :] / s_d[:, None, None]), -127, 127)
        .astype(np.int8)
        .reshape(NE, ROW)
    )
    up_q = np.ascontiguousarray(
        np.clip(np.round(up / s_u[:, None, None]), -127, 127)
        .astype(np.int8)
        .reshape(NE * RSLOT, RSLOT * DIM)
    )

    # host constants: cblob = ident [P,P] | m4g [P, NGRP*P] | e4 (padded) [P,P]
    ident_c = np.eye(P, dtype=np.float16)
    p_i = np.arange(P)[:, None, None]
    g_i = np.arange(NGRP)[None, :, None]
    col = np.arange(P)[None, None, :]
    m4g_c = (
        ((p_i // RSLOT) == (col % TPG)) & ((col // TPG) == g_i)
    ).astype(np.float16).reshape(P, NGRP * P)
    e4_pad = np.zeros((P, P), np.float16)
    e4_pad[:RSLOT, :] = (
        np.arange(RSLOT)[:, None] == (np.arange(P)[None, :] % RSLOT)
    ).astype(np.float16)


    p = np.arange(P)
    j, rp = p // RSLOT, p % RSLOT
    t_i = np.arange(N_TILE)[:, None, None]
    g_i2 = np.arange(NGRP)[None, :, None]
    toks = P * t_i + TPG * g_i2 + j[None, None, :]  # [N_TILE, NGRP, P]

    in_maps = []
    for i in range(N_CORES):
        idx_i = indices[i]  # [K]
        idxcat = np.empty((P, 2 + N_TILE * NGRP), np.int32)
        for t in range(N_TILE):
            idxcat[:, t] = idx_i[t * P : (t + 1) * P]
        up_rows = idx_i[toks] * RSLOT + rp[None, None, :]  # [N_TILE, NGRP, P]
        for t in range(N_TILE):
            for g in range(NGRP):
                idxcat[:, 2 + t * NGRP + g] = up_rows[t, g]
        cs_tok = (s_d[idx_i] * s_u[idx_i] * SCALE).astype(np.float32)  # [K]
        cs2 = np.stack([cs_tok[t * P : (t + 1) * P] for t in range(N_TILE)], axis=1)
        cblob = np.ascontiguousarray(np.concatenate([ident_c, m4g_c, e4_pad], axis=1))
        in_maps.append(
            {
                "idxcat": np.ascontiguousarray(idxcat),
                "slots": slots[i],
                "cs2": np.ascontiguousarray(cs2),
                "down": down_q,
                "up4": up_q,
                "cblob": cblob,
            }
        )
    return in_maps


def _run(in_maps, trace=False):
    from concourse.bass_utils import run_bass_kernel_spmd

    nc = _get_nc()
    return run_bass_kernel_spmd(
        nc, in_maps, core_ids=list(range(N_CORES)), trace=trace
    )


def kernel(slots, indices, down_proj_values, up_proj_values):
    in_maps = _prep_in_maps(slots, indices, down_proj_values, up_proj_values)
    res = _run(in_maps)
    out = np.stack([res.results[i]["out"] for i in range(N_CORES)], axis=0)
    return out.astype(np.float32)
